# revision 1
# baseline (speedup 1.0000x reference)
"""Trainium2 Bass kernel for nn_AttentionLayer (GN -> conv1x1 -> self-attn ->
cross-attn -> conv1x1, residuals). Data-parallel over batch: 16 samples split
across 8 NeuronCores (2 samples/core), no collectives.

Layout strategy: everything stays channel-major on chip ([C on partitions,
H*W on free]). Matmuls run in bf16 with f32 PSUM accumulation; the residual
chain is kept in f32 through h1. Self-attn softmax is computed
rows-on-partitions (exp fused on ScalarE with accumulated row sums, no
max-subtraction -- logits are provably small for this problem) with
PE-transposed attention blocks. Cross-attn is computed fully transposed
(softmax along the partition axis): per-head column sums via a tiny
ones-selector matmul, reciprocal via exp(-ln(s)) on ScalarE, broadcast back
to partitions via a selector matmul -- no per-head transposes at all.
"""

import sys

if "/opt/trn_rl_repo" not in sys.path:
    sys.path.insert(0, "/opt/trn_rl_repo")

import contextlib

import numpy as np
import ml_dtypes

import concourse.bass as bass
import concourse.mybir as mybir
from concourse import bacc
import concourse.tile as tile
from concourse.bass import ts
from concourse.bass_utils import run_bass_kernel_spmd
from concourse.masks import make_identity

BF = mybir.dt.bfloat16
F32 = mybir.dt.float32
AF = mybir.ActivationFunctionType
ALU = mybir.AluOpType
AX = mybir.AxisListType

NCORES = 8
BS = 2            # samples per core
CIN = 256         # input channels
INNER = 512       # inner channels
HW = 1024         # 32*32 spatial
CTXN = 77
CTXD = 768
HEADS = 8
DH = 64
EPS = 1e-5
SCALE_SA = float(INNER) ** -0.5   # self-attn scale (c = 512)
SCALE_CA = float(DH) ** -0.5      # cross-attn scale (1/8)

NT_CIN = CIN // 128    # 2 partition tiles of input channels
NT_IN = INNER // 128   # 4 partition tiles of inner channels
NT_HW = HW // 128      # 8 spatial tiles
NT_D = CTXD // 128     # 6 partition tiles of context dim
NH = HW // 512         # 2 free halves of spatial


def _gn_stats(nc, psB, small, sqp, x_sb, nt, gmat_sb, gexp_sb, gam_sb,
              bet_sb, inv_n, tag, eps_ap):
    """GroupNorm stats for x_sb [128, nt, 1024] f32 -> scb [128, nt, 2]
    (per-channel scale, bias')."""
    s12 = small.tile([128, nt, 2], F32, tag=f"{tag}_s12")
    sq = sqp.tile([128, 1024], F32, tag="sq_scratch")
    for ct in range(nt):
        nc.vector.tensor_reduce(out=s12[:, ct, 0:1], in_=x_sb[:, ct, :],
                                axis=AX.X, op=ALU.add)
        nc.scalar.activation(out=sq[:], in_=x_sb[:, ct, :], func=AF.Square,
                             accum_out=s12[:, ct, 1:2])
    psg = psB.tile([32, 2], F32, tag="psB")
    for ct in range(nt):
        nc.tensor.matmul(psg[:], lhsT=gmat_sb[:, ct, :], rhs=s12[:, ct, :],
                         start=(ct == 0), stop=(ct == nt - 1))
    # mm cols: 0=mu, 1=ex2, 2=rsig, 3=scratch(sig)
    mm = small.tile([32, 4], F32, tag=f"{tag}_mm")
    nc.vector.tensor_scalar(out=mm[:, 0:2], in0=psg[:], scalar1=inv_n,
                            scalar2=None, op0=ALU.mult)
    # col3 = mu*mu - ex2 = -var
    nc.vector.scalar_tensor_tensor(out=mm[:, 3:4], in0=mm[:, 0:1],
                                   scalar=mm[:, 0:1], in1=mm[:, 1:2],
                                   op0=ALU.mult, op1=ALU.subtract)
    # col3 = sqrt(-col3 + eps) = sqrt(var + eps)
    nc.scalar.activation(out=mm[:, 3:4], in_=mm[:, 3:4], func=AF.Sqrt,
                         bias=eps_ap[:32, :], scale=-1.0)
    nc.vector.reciprocal(out=mm[:, 2:3], in_=mm[:, 3:4])
    scb = small.tile([128, nt, 2], F32, tag=f"{tag}_scb")
    for ct in range(nt):
        pse = psB.tile([128, 2], F32, tag="psB")
        # expand (mu, rsig) per channel; mm[:, 0:3:2] strided view = (mu, rsig)
        nc.tensor.matmul(pse[:], lhsT=gexp_sb[:, ct, :], rhs=mm[:, 0:3:2],
                         start=True, stop=True)
        # scale = rsig_c * gamma_c
        nc.vector.tensor_tensor(out=scb[:, ct, 0:1], in0=pse[:, 1:2],
                                in1=gam_sb[:, ct:ct + 1], op=ALU.mult)
        # bias' = mu_c * scale - beta_c   (y = x*scale - bias')
        nc.vector.scalar_tensor_tensor(out=scb[:, ct, 1:2], in0=pse[:, 0:1],
                                       scalar=scb[:, ct, 0:1],
                                       in1=bet_sb[:, ct:ct + 1],
                                       op0=ALU.mult, op1=ALU.subtract)
    return scb


def _gn_apply(nc, x_sb, nt, scb, out_sb):
    for ct in range(nt):
        nc.vector.tensor_scalar(out=out_sb[:, ct, :], in0=x_sb[:, ct, :],
                                scalar1=scb[:, ct, 0:1], scalar2=scb[:, ct, 1:2],
                                op0=ALU.mult, op1=ALU.subtract)


def build(repeat=1):
    nc = bacc.Bacc("TRN2", target_bir_lowering=False, debug=False)
    d = {}

    def di(name, shape, dt):
        d[name] = nc.dram_tensor(name, shape, dt, kind="ExternalInput").ap()

    di("x", [BS, 128, NT_CIN, HW], F32)          # host pre-tiled channel-major
    di("ctx", [BS, CTXN, CTXD], BF)
    di("w_in_T", [128, NT_CIN, INNER], BF)
    di("sa_wq_T", [128, NT_IN, INNER], BF)
    di("sa_wk_T", [128, NT_IN, INNER], BF)
    di("sa_wv_T", [128, NT_IN, INNER], BF)  # holds (sa_wp @ sa_wv).T
    di("ca_wq_T", [128, NT_IN, INNER], BF)
    di("ca_wk_T", [128, NT_D, INNER], BF)
    di("ca_wv_T", [128, NT_D, INNER], BF)
    di("ca_wo_T", [128, NT_IN, INNER], BF)
    di("w_out_T", [128, NT_IN, CIN], BF)
    di("b_in", [128, NT_IN], F32)
    di("ca_bo", [128, NT_IN], F32)
    di("b_out", [128, NT_CIN], F32)
    di("gn1_g", [128, NT_CIN], F32)
    di("gn1_b", [128, NT_CIN], F32)
    di("sa_gn_g", [128, NT_IN], F32)
    di("sa_gn_b", [128, NT_IN], F32)
    di("g1mat", [128, NT_CIN, 32], F32)
    di("g1exp", [32, NT_CIN, 128], F32)
    di("g2mat", [128, NT_IN, 32], F32)
    di("g2exp", [32, NT_IN, 128], F32)
    di("sel", [HEADS, INNER], BF)
    di("emat", [CTXN, HEADS * HEADS], BF)
    out_d = nc.dram_tensor("out", [BS, CIN, HW], F32, kind="ExternalOutput").ap()

    with tile.TileContext(nc) as tc:
        with contextlib.ExitStack() as ctx:
            singles = ctx.enter_context(tc.tile_pool(name="singles", bufs=1))
            xpool = ctx.enter_context(tc.tile_pool(name="xpool", bufs=2))
            f32big = ctx.enter_context(tc.tile_pool(name="f32big", bufs=2))
            b16big = ctx.enter_context(tc.tile_pool(name="b16big", bufs=7))
            attnp = ctx.enter_context(tc.tile_pool(name="attnp", bufs=11))
            small = ctx.enter_context(tc.tile_pool(name="small", bufs=3))
            sqp = ctx.enter_context(tc.tile_pool(name="sqp", bufs=2))
            crossp = ctx.enter_context(tc.tile_pool(name="crossp", bufs=2))
            expp = ctx.enter_context(tc.tile_pool(name="expp", bufs=3))
            outp = ctx.enter_context(tc.tile_pool(name="outp", bufs=2))
            recp = ctx.enter_context(tc.tile_pool(name="recp", bufs=1))
            # PSUM: psA = 2x 2-bank tiles, psB = 2x 1-bank, psS = 1x 2-bank
            psA = ctx.enter_context(tc.tile_pool(name="psA", bufs=2, space="PSUM"))
            psB = ctx.enter_context(tc.tile_pool(name="psB", bufs=2, space="PSUM"))
            psS = ctx.enter_context(tc.tile_pool(name="psS", bufs=1, space="PSUM"))

            # ---- load weights & constants once ----
            def wload(name, shape, dt):
                t = singles.tile(shape, dt, tag=name)
                nc.sync.dma_start(out=t[:], in_=d[name])
                return t

            w_in = wload("w_in_T", [128, NT_CIN, INNER], BF)
            wq = wload("sa_wq_T", [128, NT_IN, INNER], BF)
            wk = wload("sa_wk_T", [128, NT_IN, INNER], BF)
            wv = wload("sa_wv_T", [128, NT_IN, INNER], BF)
            cwq = wload("ca_wq_T", [128, NT_IN, INNER], BF)
            cwk = wload("ca_wk_T", [128, NT_D, INNER], BF)
            cwv = wload("ca_wv_T", [128, NT_D, INNER], BF)
            cwo = wload("ca_wo_T", [128, NT_IN, INNER], BF)
            w_out = wload("w_out_T", [128, NT_IN, CIN], BF)
            b_in = wload("b_in", [128, NT_IN], F32)
            ca_bo = wload("ca_bo", [128, NT_IN], F32)
            b_out = wload("b_out", [128, NT_CIN], F32)
            gn1_g = wload("gn1_g", [128, NT_CIN], F32)
            gn1_b = wload("gn1_b", [128, NT_CIN], F32)
            gn2_g = wload("sa_gn_g", [128, NT_IN], F32)
            gn2_b = wload("sa_gn_b", [128, NT_IN], F32)
            g1mat = wload("g1mat", [128, NT_CIN, 32], F32)
            g1exp = wload("g1exp", [32, NT_CIN, 128], F32)
            g2mat = wload("g2mat", [128, NT_IN, 32], F32)
            g2exp = wload("g2exp", [32, NT_IN, 128], F32)
            sel = wload("sel", [HEADS, INNER], BF)
            emat = wload("emat", [CTXN, HEADS * HEADS], BF)
            id_bf = singles.tile([128, 128], BF, tag="id_bf")
            make_identity(nc, id_bf[:])
            eps_t = singles.tile([128, 1], F32, tag="eps")
            nc.gpsimd.memset(eps_t[:], EPS)
            ones128 = singles.tile([128, 1], BF, tag="ones128")
            nc.gpsimd.memset(ones128[:], 1.0)
            ones1 = singles.tile([1, 128], BF, tag="ones1")
            nc.gpsimd.memset(ones1[:], 1.0)

            def phase_A(s, st):
                # GN1 + conv_in
                x_sb = xpool.tile([128, NT_CIN, HW], BF, tag="x")
                nc.gpsimd.dma_start(out=x_sb[:], in_=d["x"][s])
                gn1 = b16big.tile([128, NT_IN, HW], BF, tag="big16")
                scb = _gn_stats(nc, psB, small, sqp, x_sb, NT_CIN, g1mat, g1exp,
                                gn1_g, gn1_b, 1.0 / (8 * HW), "gn1", eps_t)
                _gn_apply(nc, x_sb, NT_CIN, scb, gn1)
                h0 = f32big.tile([128, NT_IN, HW], F32, tag="f32big")
                for m in range(NT_IN):
                    ps = psA.tile([128, HW], F32, tag="psA")
                    for h in range(NH):
                        for c in range(NT_CIN):
                            nc.tensor.matmul(ps[:, ts(h, 512)],
                                             lhsT=w_in[:, c, ts(m, 128)],
                                             rhs=gn1[:, c, ts(h, 512)],
                                             start=(c == 0), stop=(c == NT_CIN - 1))
                    nc.vector.tensor_scalar(out=h0[:, m, :], in0=ps[:],
                                            scalar1=b_in[:, m:m + 1], scalar2=None,
                                            op0=ALU.add)
                st["x_sb"], st["h0"] = x_sb, h0

            def phase_Bstats(s, st):
                st["scb2"] = _gn_stats(nc, psB, small, sqp, st["h0"], NT_IN, g2mat,
                                       g2exp, gn2_g, gn2_b, 1.0 / (16 * HW),
                                       "gn2", eps_t)

            def phase_Bqkv(s, st):
                gn2 = b16big.tile([128, NT_IN, HW], BF, tag="big16")
                _gn_apply(nc, st["h0"], NT_IN, st["scb2"], gn2)
                q_sb = b16big.tile([128, NT_IN, HW], BF, tag="big16")
                k_sb = b16big.tile([128, NT_IN, HW], BF, tag="big16")
                for dst, w in ((q_sb, wq), (k_sb, wk)):
                    for m in range(NT_IN):
                        ps = psA.tile([128, HW], F32, tag="psA")
                        for h in range(NH):
                            for c in range(NT_IN):
                                nc.tensor.matmul(ps[:, ts(h, 512)],
                                                 lhsT=w[:, c, ts(m, 128)],
                                                 rhs=gn2[:, c, ts(h, 512)],
                                                 start=(c == 0),
                                                 stop=(c == NT_IN - 1))
                        nc.scalar.copy(out=dst[:, m, :], in_=ps[:])
                # vT directly: stationary = gn2 tile, moving = (wp@wv).T block
                vT = b16big.tile([128, NT_HW, 512], BF, tag="big16")
                for jb in range(NT_HW):
                    ps = psB.tile([128, 512], F32, tag="psB")
                    for c in range(NT_IN):
                        nc.tensor.matmul(ps[:], lhsT=gn2[:, c, ts(jb, 128)],
                                         rhs=wv[:, c, :],
                                         start=(c == 0), stop=(c == NT_IN - 1))
                    nc.scalar.copy(out=vT[:, jb, :], in_=ps[:])
                st["q"], st["k"], st["vT"] = q_sb, k_sb, vT

            def phase_C(s, st):
                # transposed self-attention: simT = k^T q; softmax along
                # partitions (column sums via ones-matmul, recip via
                # exp(-ln(s)), broadcast via ones-column matmul); v carries
                # sa_wp so o == wp-proj, channel-major, no transposes.
                q_sb, k_sb, vT, h0 = st["q"], st["k"], st["vT"], st["h0"]
                h1b = b16big.tile([128, NT_IN, HW], BF, tag="big16")
                ssums = psS.tile([HEADS, HW], F32, tag="psS")
                eTs = []
                for jb in range(NT_HW):
                    ps_sim = psA.tile([128, HW], F32, tag="psA")
                    for h in range(NH):
                        for c in range(NT_IN):
                            nc.tensor.matmul(ps_sim[:, ts(h, 512)],
                                             lhsT=k_sb[:, c, ts(jb, 128)],
                                             rhs=q_sb[:, c, ts(h, 512)],
                                             start=(c == 0), stop=(c == NT_IN - 1))
                    eT = attnp.tile([128, HW], BF, tag="eT")
                    nc.scalar.activation(out=eT[:], in_=ps_sim[:], func=AF.Exp,
                                         scale=SCALE_SA)
                    for h in range(NH):
                        nc.tensor.matmul(ssums[0:1, ts(h, 512)],
                                         lhsT=ones128[:], rhs=eT[:, ts(h, 512)],
                                         start=(jb == 0), stop=(jb == NT_HW - 1))
                    eTs.append(eT)
                recS = recp.tile([1, HW], BF, tag="recS")
                nc.scalar.activation(out=recS[:], in_=ssums[0:1, :], func=AF.Ln)
                nc.scalar.activation(out=recS[:], in_=recS[:], func=AF.Exp,
                                     scale=-1.0)
                recB = crossp.tile([128, HW], BF, tag="rB")
                for h in range(NH):
                    ps_rb = psB.tile([128, 512], F32, tag="psB")
                    nc.tensor.matmul(ps_rb[:], lhsT=ones1[:],
                                     rhs=recS[0:1, ts(h, 512)],
                                     start=True, stop=True)
                    nc.scalar.copy(out=recB[:, ts(h, 512)], in_=ps_rb[:])
                for c2 in range(NT_IN):
                    ps_o = psA.tile([128, HW], F32, tag="psA")
                    for h in range(NH):
                        for jb in range(NT_HW):
                            nc.tensor.matmul(ps_o[:, ts(h, 512)],
                                             lhsT=vT[:, jb, ts(c2, 128)],
                                             rhs=eTs[jb][:, ts(h, 512)],
                                             start=(jb == 0), stop=(jb == NT_HW - 1))
                    tmp = attnp.tile([128, HW], BF, tag="eT")
                    nc.vector.tensor_tensor(out=tmp[:], in0=ps_o[:], in1=recB[:],
                                            op=ALU.mult)
                    # double residual: h1 = 2*h0 + proj (bf16 master)
                    nc.vector.scalar_tensor_tensor(
                        out=h1b[:, c2, :], in0=h0[:, c2, :], scalar=2.0,
                        in1=tmp[:], op0=ALU.mult, op1=ALU.add)
                st["h1b"] = h1b

            def phase_E(s, st):
                h1b = st["h1b"]
                ctx_sb = crossp.tile([CTXN, CTXD], BF, tag="ctx")
                nc.sync.dma_start(out=ctx_sb[:], in_=d["ctx"][s])
                # pad per-block stride to 80 so bf16 PSUM offsets stay aligned
                psT3 = psB.tile([128, NT_D, 80], BF, tag="psB")
                for dd in range(NT_D):
                    nc.tensor.transpose(psT3[:, dd, :CTXN], ctx_sb[:, ts(dd, 128)],
                                        id_bf[:CTXN, :CTXN])
                ctxT = crossp.tile([128, NT_D, CTXN], BF, tag="ctxT")
                nc.scalar.copy(out=ctxT[:], in_=psT3[:, :, :CTXN])
                # kT [512, 77]
                ps_kt = psB.tile([128, NT_IN, CTXN], F32, tag="psB")
                for m in range(NT_IN):
                    for dd in range(NT_D):
                        nc.tensor.matmul(ps_kt[:, m, :],
                                         lhsT=cwk[:, dd, ts(m, 128)],
                                         rhs=ctxT[:, dd, :],
                                         start=(dd == 0), stop=(dd == NT_D - 1))
                kT = crossp.tile([128, NT_IN, CTXN], BF, tag="kT")
                nc.scalar.copy(out=kT[:], in_=ps_kt[:])
                # v [77, 512]
                ps_v = psB.tile([CTXN, 512], F32, tag="psB")
                for dd in range(NT_D):
                    nc.tensor.matmul(ps_v[:], lhsT=ctxT[:, dd, :], rhs=cwv[:, dd, :],
                                     start=(dd == 0), stop=(dd == NT_D - 1))
                v_sb = crossp.tile([CTXN, 512], BF, tag="v_sb")
                nc.scalar.copy(out=v_sb[:], in_=ps_v[:])
                # qT [512, 1024]
                qT = b16big.tile([128, NT_IN, HW], BF, tag="big16")
                for m in range(NT_IN):
                    ps = psA.tile([128, HW], F32, tag="psA")
                    for h in range(NH):
                        for c in range(NT_IN):
                            nc.tensor.matmul(ps[:, ts(h, 512)],
                                             lhsT=cwq[:, c, ts(m, 128)],
                                             rhs=h1b[:, c, ts(h, 512)],
                                             start=(c == 0), stop=(c == NT_IN - 1))
                    nc.scalar.copy(out=qT[:, m, :], in_=ps[:])
                st["kT"], st["v"], st["qT"] = kT, v_sb, qT

            def phase_F(s, st):
                # transposed cross-attention (see phase C comments)
                kT, v_sb, qT = st["kT"], st["v"], st["qT"]
                oxT = b16big.tile([128, NT_IN, HW], BF, tag="big16")
                hsums = psS.tile([HEADS, HW], F32, tag="psS")
                for ct in range(NT_IN):
                    eTs = []
                    for hh in range(2):
                        hd = 2 * ct + hh
                        po = (hd % 2) * 64
                        mt = hd // 2
                        ps_sT = psA.tile([CTXN, HW], F32, tag="psA")
                        for h in range(NH):
                            nc.tensor.matmul(ps_sT[:, ts(h, 512)],
                                             lhsT=kT[po:po + 64, mt, :],
                                             rhs=qT[po:po + 64, mt, ts(h, 512)],
                                             start=True, stop=True)
                        eT = expp.tile([CTXN, HW], BF, tag="expT")
                        nc.scalar.activation(out=eT[:], in_=ps_sT[:],
                                             func=AF.Exp, scale=SCALE_CA)
                        for h in range(NH):
                            nc.tensor.matmul(hsums[:, ts(h, 512)],
                                             lhsT=emat[:, ts(hd, HEADS)],
                                             rhs=eT[:, ts(h, 512)],
                                             start=(hd == 0), stop=(hd == HEADS - 1))
                        eTs.append(eT)
                    ps_or = psA.tile([128, HW], F32, tag="psA")
                    for hh in range(2):
                        hd = 2 * ct + hh
                        for h in range(NH):
                            nc.tensor.matmul(ps_or[hh * 64:hh * 64 + 64, ts(h, 512)],
                                             lhsT=v_sb[:, ts(hd, DH)],
                                             rhs=eTs[hh][:, ts(h, 512)],
                                             start=True, stop=True)
                    nc.scalar.copy(out=oxT[:, ct, :], in_=ps_or[:])
                # reciprocal of sums: exp(-ln(s)) on ScalarE
                rec = recp.tile([HEADS, HW], BF, tag="rec")
                nc.scalar.activation(out=rec[:], in_=hsums[:], func=AF.Ln)
                nc.scalar.activation(out=rec[:], in_=rec[:], func=AF.Exp,
                                     scale=-1.0)
                for ct in range(NT_IN):
                    rB = crossp.tile([128, HW], BF, tag="rB")
                    for h in range(NH):
                        ps_rb = psB.tile([128, 512], F32, tag="psB")
                        nc.tensor.matmul(ps_rb[:], lhsT=sel[:, ts(ct, 128)],
                                         rhs=rec[:, ts(h, 512)],
                                         start=True, stop=True)
                        nc.scalar.copy(out=rB[:, ts(h, 512)], in_=ps_rb[:])
                    nc.vector.tensor_tensor(out=oxT[:, ct, :],
                                            in0=oxT[:, ct, :], in1=rB[:],
                                            op=ALU.mult)
                st["oxT"] = oxT

            def phase_IJ(s, st):
                oxT, h1b, x_sb = st["oxT"], st["h1b"], st["x_sb"]
                h2b = b16big.tile([128, NT_IN, HW], BF, tag="big16")
                for m in range(NT_IN):
                    ps = psA.tile([128, HW], F32, tag="psA")
                    for h in range(NH):
                        for c in range(NT_IN):
                            nc.tensor.matmul(ps[:, ts(h, 512)],
                                             lhsT=cwo[:, c, ts(m, 128)],
                                             rhs=oxT[:, c, ts(h, 512)],
                                             start=(c == 0), stop=(c == NT_IN - 1))
                    nc.vector.scalar_tensor_tensor(out=h2b[:, m, :], in0=ps[:],
                                                   scalar=ca_bo[:, m:m + 1],
                                                   in1=h1b[:, m, :],
                                                   op0=ALU.add, op1=ALU.add)
                for m in range(NT_CIN):
                    ps = psA.tile([128, HW], F32, tag="psA")
                    for h in range(NH):
                        for c in range(NT_IN):
                            nc.tensor.matmul(ps[:, ts(h, 512)],
                                             lhsT=w_out[:, c, ts(m, 128)],
                                             rhs=h2b[:, c, ts(h, 512)],
                                             start=(c == 0), stop=(c == NT_IN - 1))
                    ot = outp.tile([128, HW], F32, tag="outt")
                    nc.vector.scalar_tensor_tensor(out=ot[:], in0=ps[:],
                                                   scalar=b_out[:, m:m + 1],
                                                   in1=x_sb[:, m, :],
                                                   op0=ALU.add, op1=ALU.add)
                    nc.sync.dma_start(out=out_d[s, ts(m, 128)], in_=ot[:])

            # Interleaved schedule: sample 1's GN/conv work is traced early so
            # the scheduler can hide it under sample 0's PE-heavy phases.
            for _ in range(repeat):
                st = [dict(), dict()]
                phase_A(0, st[0])
                phase_Bstats(0, st[0])
                phase_Bqkv(0, st[0])
                phase_A(1, st[1])
                phase_C(0, st[0])
                phase_Bstats(1, st[1])
                phase_E(0, st[0])
                phase_F(0, st[0])
                phase_IJ(0, st[0])
                phase_Bqkv(1, st[1])
                phase_C(1, st[1])
                phase_E(1, st[1])
                phase_F(1, st[1])
                phase_IJ(1, st[1])

    nc.compile()
    return nc


# ---------------------------------------------------------------------------
# host-side wrapper
# ---------------------------------------------------------------------------

def _tile_rows(a, dt):
    """[R, M] -> [128, R//128, M] partition-tiled, contiguous."""
    r, m = a.shape
    return np.ascontiguousarray(
        a.reshape(r // 128, 128, m).transpose(1, 0, 2).astype(dt))


def _col_tiled(v, dt=np.float32):
    """[C] -> [128, C//128]."""
    c = v.shape[0]
    return np.ascontiguousarray(v.reshape(c // 128, 128).T.astype(dt))


def prep_inputs(inputs):
    bf = ml_dtypes.bfloat16
    f32 = np.float32
    x = np.asarray(inputs["x"], f32).reshape(NCORES, BS, CIN, HW)
    # [core, s, 256, 1024] -> [core, s, 128, 2, 1024]
    x = np.ascontiguousarray(
        x.reshape(NCORES, BS, NT_CIN, 128, HW).transpose(0, 1, 3, 2, 4))
    ctxa = np.asarray(inputs["context"], f32).astype(bf).reshape(
        NCORES, BS, CTXN, CTXD)

    g1mat = np.zeros((CIN, 32), f32)
    g1mat[np.arange(CIN), np.arange(CIN) // 8] = 1.0
    g2mat = np.zeros((INNER, 32), f32)
    g2mat[np.arange(INNER), np.arange(INNER) // 16] = 1.0
    sel = np.zeros((HEADS, INNER), f32)
    sel[np.arange(INNER) // DH, np.arange(INNER)] = 1.0
    emat = np.zeros((CTXN, HEADS * HEADS), f32)
    for hd in range(HEADS):
        emat[:, hd * HEADS + hd] = 1.0

    com = {
        "w_in_T": _tile_rows(np.asarray(inputs["w_in"], f32).T, bf),
        "sa_wq_T": _tile_rows(np.asarray(inputs["sa_wq"], f32).T, bf),
        "sa_wk_T": _tile_rows(np.asarray(inputs["sa_wk"], f32).T, bf),
        "sa_wv_T": _tile_rows(
            (np.asarray(inputs["sa_wp"], f32) @
             np.asarray(inputs["sa_wv"], f32)).T, bf),
        "ca_wq_T": _tile_rows(np.asarray(inputs["ca_wq"], f32).T, bf),
        "ca_wk_T": _tile_rows(np.asarray(inputs["ca_wk"], f32).T, bf),
        "ca_wv_T": _tile_rows(np.asarray(inputs["ca_wv"], f32).T, bf),
        "ca_wo_T": _tile_rows(np.asarray(inputs["ca_wo"], f32).T, bf),
        "w_out_T": _tile_rows(np.asarray(inputs["w_out"], f32).T, bf),
        "b_in": _col_tiled(np.asarray(inputs["b_in"], f32)),
        "ca_bo": _col_tiled(np.asarray(inputs["ca_bo"], f32)),
        "b_out": _col_tiled(np.asarray(inputs["b_out"], f32)),
        "gn1_g": _col_tiled(np.asarray(inputs["gn1_g"], f32)),
        "gn1_b": _col_tiled(np.asarray(inputs["gn1_b"], f32)),
        "sa_gn_g": _col_tiled(np.asarray(inputs["sa_gn_g"], f32)),
        "sa_gn_b": _col_tiled(np.asarray(inputs["sa_gn_b"], f32)),
        "g1mat": _tile_rows(g1mat, f32),
        "g1exp": np.ascontiguousarray(
            g1mat.T.reshape(32, NT_CIN, 128).astype(f32)),
        "g2mat": _tile_rows(g2mat, f32),
        "g2exp": np.ascontiguousarray(
            g2mat.T.reshape(32, NT_IN, 128).astype(f32)),
        "sel": sel.astype(bf),
        "emat": emat.astype(bf),
    }
    return [{**com, "x": np.ascontiguousarray(x[c]),
             "ctx": np.ascontiguousarray(ctxa[c])} for c in range(NCORES)]


def assemble_output(results):
    # results: list (per core) of {"out": [BS, 256, 1024]}
    outs = np.stack([r["out"] for r in results])      # [8, 2, 256, 1024]
    return outs.reshape(16, CIN, 32, 32)


_CACHE = {}


def kernel(**inputs) -> np.ndarray:
    if "nc" not in _CACHE:
        _CACHE["nc"] = build(repeat=1)
    nc = _CACHE["nc"]
    in_maps = prep_inputs(inputs)
    res = run_bass_kernel_spmd(nc, in_maps, core_ids=list(range(NCORES)))
    return assemble_output(res.results)


# ---------------------------------------------------------------------------
# device-resident runner (for timing): keeps inputs on device, feeds outputs
# back as donated output buffers so repeated calls ship no data.
# ---------------------------------------------------------------------------

class DeviceRunner:
    def __init__(self, nc):
        import jax
        from jax.sharding import Mesh, PartitionSpec, NamedSharding
        from jax.experimental.shard_map import shard_map
        from concourse.bass2jax import (_bass_exec_p, install_neuronx_cc_hook,
                                        partition_id_tensor)
        install_neuronx_cc_hook()
        self.jax = jax
        self.nc = nc
        pname = nc.partition_id_tensor.name if nc.partition_id_tensor else None
        in_names, out_names, out_avals, zero_outs = [], [], [], []
        for alloc in nc.m.functions[0].allocations:
            if not isinstance(alloc, mybir.MemoryLocationSet):
                continue
            name = alloc.memorylocations[0].name
            if alloc.kind == "ExternalInput":
                if name != pname:
                    in_names.append(name)
            elif alloc.kind == "ExternalOutput":
                out_names.append(name)
                shape = tuple(alloc.tensor_shape)
                dtype = mybir.dt.np(alloc.dtype)
                out_avals.append(jax.core.ShapedArray(shape, dtype))
                zero_outs.append(np.zeros(shape, dtype))
        self.in_names, self.out_names = in_names, out_names
        self.out_avals, self.zero_outs = out_avals, zero_outs
        n_params, n_outs = len(in_names), len(out_avals)
        names_all = in_names + out_names + ([pname] if pname else [])

        def _body(*args):
            operands = list(args)
            if pname is not None:
                operands.append(partition_id_tensor())
            return tuple(_bass_exec_p.bind(
                *operands, out_avals=tuple(out_avals),
                in_names=tuple(names_all), out_names=tuple(out_names),
                lowering_input_output_aliases=(), sim_require_finite=True,
                sim_require_nnan=True, nc=nc))

        devices = jax.devices()[:NCORES]
        self.mesh = Mesh(np.asarray(devices), ("core",))
        self.sh = NamedSharding(self.mesh, PartitionSpec("core"))
        self.fn = jax.jit(
            shard_map(_body, mesh=self.mesh,
                      in_specs=(PartitionSpec("core"),) * (n_params + n_outs),
                      out_specs=(PartitionSpec("core"),) * n_outs,
                      check_rep=False),
            donate_argnums=tuple(range(n_params, n_params + n_outs)),
            keep_unused=True)

    def put_inputs(self, in_maps):
        jax = self.jax
        concat = [np.concatenate([np.asarray(m[n]) for m in in_maps], axis=0)
                  for n in self.in_names]
        self.in_dev = [jax.device_put(a, self.sh) for a in concat]
        self.outs = self.fn(*self.in_dev, *[
            jax.device_put(np.zeros((NCORES * z.shape[0], *z.shape[1:]), z.dtype),
                           self.sh) for z in self.zero_outs])
        jax.block_until_ready(self.outs)

    def run_once(self):
        self.outs = self.fn(*self.in_dev, *self.outs)
        return self.outs

    def time_iters(self, iters):
        import time as _t
        jax = self.jax
        t0 = _t.perf_counter()
        for _ in range(iters):
            self.outs = self.fn(*self.in_dev, *self.outs)
        jax.block_until_ready(self.outs)
        return (_t.perf_counter() - t0) / iters

    def get_outputs(self):
        res = [np.asarray(o) for o in self.jax.block_until_ready(self.outs)]
        per_core = []
        for c in range(NCORES):
            m = {}
            for i, nme in enumerate(self.out_names):
                shp = self.out_avals[i].shape
                m[nme] = res[i].reshape(NCORES, *shp)[c]
            per_core.append(m)
        return per_core

